# revision 1
# baseline (speedup 1.0000x reference)
"""Trainium2 Bass kernel for single-head attention with query-axis softmax.

Problem (B=4, S=2048, D=1024):
    q = seq1 @ Wq^T ; k = seq2 @ Wk^T ; v = seq2 @ Wv^T
    score = q @ k^T / sqrt(D)
    mask_score = where(attn_mask, 1e-9, score)
    p = softmax(mask_score, axis=1)          # softmax over the QUERY axis
    out = p @ v

Math used here: softmax over q means p[q,k] = exp(s[q,k]) / Z[k] with
Z[k] = sum_q exp(s[q,k]) (no max-subtraction needed: |s| <= ~3, and
exp(1e-9) == 1.0f == exp(0.0) in fp32, so masked entries are exactly
reproduced by zeroing the score). Then
    out = E @ (v / Z)  with E = exp(s_masked).

Sharding: 8 cores = 4 batches x 2 key-halves. Each core computes the
partial out for its key half; the host sums the two halves per batch.
Scores are built TRANSPOSED (k on partitions, q on the free axis) so the
query-axis softmax is a free-axis reduction fused into the Exp activation.
"""

import numpy as np
import ml_dtypes

import concourse.bass as bass
import concourse.tile as tile
from concourse import bacc, mybir
from concourse import bass_utils

B, S, D = 4, 2048, 1024
KSPLIT = 2
KH = S // KSPLIT            # 1024 keys per core
P = 128                     # partitions
DC = D // P                 # 8 contraction chunks (d)
HC = D // P                 # 8 hidden chunks (h)
KC = KH // P                # 8 key chunks
QN = S // 512               # 4 q tiles of 512
KN = KH // 512              # 2 k tiles of 512
HN = D // 512               # 2 h tiles of 512

BF16 = mybir.dt.bfloat16
F32 = mybir.dt.float32
U8 = mybir.dt.uint8

_NC = None


def _emit(nc):
    import contextlib

    s1t = nc.dram_tensor("s1t", [D, S], BF16, kind="ExternalInput").ap()
    s2t = nc.dram_tensor("s2t", [D, KH], BF16, kind="ExternalInput").ap()
    wqt = nc.dram_tensor("wqt", [D, D], BF16, kind="ExternalInput").ap()
    wkt = nc.dram_tensor("wkt", [D, D], BF16, kind="ExternalInput").ap()
    wvt = nc.dram_tensor("wvt", [D, D], BF16, kind="ExternalInput").ap()
    nmk = nc.dram_tensor("nmk", [KH, S], U8, kind="ExternalInput").ap()
    out = nc.dram_tensor("out", [S, D], F32, kind="ExternalOutput").ap()

    # HBM views with 128-partition chunking
    s1t_v = s1t.rearrange("(c p) q -> p c q", p=P)
    s2t_v = s2t.rearrange("(c p) k -> p c k", p=P)
    wqt_v = wqt.rearrange("(c p) h -> p c h", p=P)
    wkt_v = wkt.rearrange("(c p) h -> p c h", p=P)
    wvt_v = wvt.rearrange("(c p) h -> p c h", p=P)
    nmk_v = nmk.rearrange("(c p) q -> p c q", p=P)
    out_v = out.rearrange("(c p) h -> p c h", p=P)

    with tile.TileContext(nc) as tc, contextlib.ExitStack() as ctx:
        wpool = ctx.enter_context(tc.tile_pool(name="wpool", bufs=1))
        big = ctx.enter_context(tc.tile_pool(name="big", bufs=1))
        mid = ctx.enter_context(tc.tile_pool(name="mid", bufs=1))
        small = ctx.enter_context(tc.tile_pool(name="small", bufs=1))
        ostp = ctx.enter_context(tc.tile_pool(name="ostp", bufs=3))
        psum = ctx.enter_context(tc.tile_pool(name="psum", bufs=8, space="PSUM"))

        # ---- resident SBUF tensors ----
        wq_sb = wpool.tile([P, DC, D], BF16)
        wk_sb = wpool.tile([P, DC, D], BF16)
        wv_sb = wpool.tile([P, DC, D], BF16)
        s1_sb = big.tile([P, DC, S], BF16, tag="bigA")      # seq1^T  [d, q]
        s2_sb = mid.tile([P, DC, KH], BF16, tag="midA")     # seq2^T  [d, k]
        nm_sb = small.tile([P, KC, S], U8)                  # notmask [k, q]
        qt_sb = small.tile([P, HC, S], BF16)                # Q^T     [h, q]
        kt_sb = small.tile([P, HC, KH], BF16)               # K^T     [h, k]
        v_sb = small.tile([P, KC, D], BF16)                 # V       [k, h]
        z4_sb = small.tile([P, KC, QN], F32)
        z_sb = small.tile([P, KC], F32)
        rz_sb = small.tile([P, KC], F32)
        # E shares the slot of s1 (s1 is dead once QT is computed);
        # vpp shares the slot of s2 (dead once KT2/V are computed).
        e_sb = big.tile([P, KC, S], BF16, tag="bigA")       # E       [k, q]
        vpp_sb = mid.tile([P, KC, D], BF16, tag="midA")     # V/Z     [k, h]

        # ---- loads ----
        for c in range(DC):
            nc.sync.dma_start(out=s2_sb[:, c, :], in_=s2t_v[:, c, :])
        for c in range(DC):
            nc.sync.dma_start(out=wk_sb[:, c, :], in_=wkt_v[:, c, :])
            nc.sync.dma_start(out=wv_sb[:, c, :], in_=wvt_v[:, c, :])
        for c in range(DC):
            nc.sync.dma_start(out=s1_sb[:, c, :], in_=s1t_v[:, c, :])
            nc.sync.dma_start(out=wq_sb[:, c, :], in_=wqt_v[:, c, :])
        for c in range(KC):
            nc.sync.dma_start(out=nm_sb[:, c, :], in_=nmk_v[:, c, :])

        # ---- KT2[h, k] = Wk @ seq2^T : lhsT=wkt chunk, rhs=s2t ----
        for hc in range(HC):
            for kt in range(KN):
                ps = psum.tile([P, 512], F32, tag="ps")
                for dc in range(DC):
                    nc.tensor.matmul(
                        ps,
                        wk_sb[:, dc, hc * P:(hc + 1) * P],
                        s2_sb[:, dc, kt * 512:(kt + 1) * 512],
                        start=(dc == 0), stop=(dc == DC - 1),
                    )
                nc.scalar.copy(out=kt_sb[:, hc, kt * 512:(kt + 1) * 512], in_=ps)

        # ---- V[k, h] = seq2 @ Wv^T : lhsT=s2t chunk, rhs=wvt ----
        for kc in range(KC):
            for ht in range(HN):
                ps = psum.tile([P, 512], F32, tag="ps")
                for dc in range(DC):
                    nc.tensor.matmul(
                        ps,
                        s2_sb[:, dc, kc * P:(kc + 1) * P],
                        wv_sb[:, dc, ht * 512:(ht + 1) * 512],
                        start=(dc == 0), stop=(dc == DC - 1),
                    )
                nc.scalar.copy(out=v_sb[:, kc, ht * 512:(ht + 1) * 512], in_=ps)

        # ---- QT[h, q] = (Wq/32) @ seq1^T : lhsT=wqt chunk, rhs=s1t ----
        # (qt outer so the score phase can start as soon as a q stripe is done)
        for qt in range(QN):
            for hc in range(HC):
                ps = psum.tile([P, 512], F32, tag="ps")
                for dc in range(DC):
                    nc.tensor.matmul(
                        ps,
                        wq_sb[:, dc, hc * P:(hc + 1) * P],
                        s1_sb[:, dc, qt * 512:(qt + 1) * 512],
                        start=(dc == 0), stop=(dc == DC - 1),
                    )
                nc.vector.tensor_copy(out=qt_sb[:, hc, qt * 512:(qt + 1) * 512], in_=ps)

        # ---- sT[k, q] = KT2^T-contract-h @ QT ; mask ; exp ; Z ----
        for kc in range(KC):
            for qt in range(QN):
                ps = psum.tile([P, 512], F32, tag="ps")
                for hc in range(HC):
                    nc.tensor.matmul(
                        ps,
                        kt_sb[:, hc, kc * P:(kc + 1) * P],
                        qt_sb[:, hc, qt * 512:(qt + 1) * 512],
                        start=(hc == 0), stop=(hc == HC - 1),
                    )
                # masked scores -> 0 (exp -> 1.0 == fp32 exp(1e-9))
                nc.vector.tensor_mul(ps, ps, nm_sb[:, kc, qt * 512:(qt + 1) * 512])
                nc.scalar.activation(
                    out=e_sb[:, kc, qt * 512:(qt + 1) * 512],
                    in_=ps,
                    func=mybir.ActivationFunctionType.Exp,
                    accum_out=z4_sb[:, kc, qt:qt + 1],
                )
            # Z[k] = sum_q E ; vpp = V / Z
            nc.vector.reduce_sum(out=z_sb[:, kc:kc + 1], in_=z4_sb[:, kc, :],
                                 axis=mybir.AxisListType.X)
            nc.vector.reciprocal(rz_sb[:, kc:kc + 1], z_sb[:, kc:kc + 1])
            nc.vector.tensor_scalar_mul(vpp_sb[:, kc, :], v_sb[:, kc, :],
                                        rz_sb[:, kc:kc + 1])

        # ---- out[q, h] = E^T-contract-k @ vpp ----
        for qc in range(S // P):
            ost = ostp.tile([P, D], F32, tag="ost")
            for ht in range(HN):
                ps = psum.tile([P, 512], F32, tag="ps")
                for kc in range(KC):
                    nc.tensor.matmul(
                        ps,
                        e_sb[:, kc, qc * P:(qc + 1) * P],
                        vpp_sb[:, kc, ht * 512:(ht + 1) * 512],
                        start=(kc == 0), stop=(kc == KC - 1),
                    )
                nc.scalar.copy(out=ost[:, ht * 512:(ht + 1) * 512], in_=ps)
            nc.sync.dma_start(out=out_v[:, qc, :], in_=ost)


def _build():
    nc = bacc.Bacc("TRN2", target_bir_lowering=False, debug=False,
                   enable_asserts=False)
    _emit(nc)
    nc.compile()
    return nc


def _get_nc():
    global _NC
    if _NC is None:
        _NC = _build()
    return _NC


def _prep_inputs(seq1, seq2, attn_mask, Wq, Wk, Wv):
    bf16 = ml_dtypes.bfloat16
    seq1 = np.asarray(seq1, dtype=np.float32)
    seq2 = np.asarray(seq2, dtype=np.float32)
    attn_mask = np.asarray(attn_mask).astype(bool)
    # fold the 1/sqrt(D) score scale into Wq before the bf16 cast
    wqt_h = np.ascontiguousarray((np.asarray(Wq, np.float32) / np.sqrt(np.float32(D))).T).astype(bf16)
    wkt_h = np.ascontiguousarray(np.asarray(Wk, np.float32).T).astype(bf16)
    wvt_h = np.ascontiguousarray(np.asarray(Wv, np.float32).T).astype(bf16)

    in_maps = []
    for c in range(8):
        b, khalf = divmod(c, KSPLIT)
        ks, ke = khalf * KH, (khalf + 1) * KH
        in_maps.append({
            "s1t": np.ascontiguousarray(seq1[b].T).astype(bf16),
            "s2t": np.ascontiguousarray(seq2[b, ks:ke, :].T).astype(bf16),
            "wqt": wqt_h,
            "wkt": wkt_h,
            "wvt": wvt_h,
            "nmk": np.ascontiguousarray((~attn_mask[b, :, ks:ke]).T).astype(np.uint8),
        })
    return in_maps


def kernel(seq1, seq2, attn_mask, Wq, Wk, Wv):
    nc = _get_nc()
    in_maps = _prep_inputs(seq1, seq2, attn_mask, Wq, Wk, Wv)
    res = bass_utils.run_bass_kernel_spmd(nc, in_maps, core_ids=list(range(8)))
    out = np.zeros((B, S, D), np.float32)
    for c in range(8):
        out[c // KSPLIT] += res.results[c]["out"]
    return out
